# revision 35
# baseline (speedup 1.0000x reference)
"""Vocab-parallel MEVO softmax-cross-entropy loss kernel for 8 Trainium2 cores.

Strategy (vocab-parallel, per sharding hint):
  - proj_weight is sharded row-wise (vocab dim) across 8 cores: 4000 rows each.
  - Tokens are sorted by target id on the host (pure index manipulation); the
    same permuted token order is used by every core, so per-core outputs line
    up and the final token sum is order-invariant.
  - Each core computes logits = x @ Wc^T for its vocab shard in fp8-e4m3 with
    DoubleRow perf mode (2 contraction slabs per matmul, 0.5 cyc/row on the
    PE) accumulating in fp32 PSUM. Inputs are pre-scaled by 64 on the host so
    fp8 values sit in e4m3's normal range; the 64*64=4096 logit scale is
    removed inside the exp (ACT scale=1/4096) and on the host for the target
    scores.
  - exp+row-sum is fused on the scalar engine via activation(accum_out=...)
    (no explicit max: logits are O(0.1), exp cannot overflow, and
    log(sum(exp(l))) == max + log(sum(exp(l-max))) exactly).
  - The target score logit[t, tgt_t] is extracted for tokens whose target
    lives in this core's shard with one fused DVE op per masked tile:
    (iota == local_target) * logits, row-sum accumulated. Because tokens are
    target-sorted, only ~2 tiles per token tile contain an owned target, so
    this is ~3% of tiles (the masked set is computed exactly from the data at
    build time and is identical across cores; non-owned tokens carry a -1e9
    sentinel that never matches).
  - Host epilogue: S_t = sum_c s_ct ; loss = sum_t log(S_t) - sum tscore.
"""

import numpy as np
import ml_dtypes

TOKENS, D, VOCAB, NCORES = 8192, 1024, 32000, 8
VS = VOCAB // NCORES      # 4000 vocab rows per core
NT = 500                  # vocab free-dim tile (500 f32 = 2000B = one PSUM bank)
NJ = VS // NT             # 8 vocab tiles
TOK_TILE = 128
NI = TOKENS // TOK_TILE   # 64 token tiles
NK = D // 128             # 8 contraction slabs of 128
SCALE = 64.0              # per-input fp8 scale; logits carry SCALE**2
# PSUM group layout: list of (n_banks, kind); kinds: 'A' = ACT exp,
# 'D' = DVE cubic-Taylor. Bank counts must sum to <= 8.
GROUPS = [(1, "D"), (1, "A"), (2, "A"), (2, "A"), (2, "A")]
FLIP_EVERY = 4            # every Nth token tile, position 1 flips A->D (0=off)
FLIP2_EVERY = 0           # every Nth tile (offset 1), position 2 flips A->D (off)
PBUFS = 1                 # buffers per group position (positions self-pipeline)
DESCALE = 1.0 / (SCALE * SCALE)

_CACHE = {}


def _build(masked):
    """Build the single SPMD Bass program. `masked` = sorted tuple of (i, j)
    token-tile/vocab-tile pairs that need target-score extraction (union over
    cores)."""
    import concourse.mybir as mybir
    import concourse.tile as tile
    from concourse import bacc
    from concourse.bass import ts, ds

    f32 = mybir.dt.float32
    fp8 = mybir.dt.float8e4
    NM = max(len(masked), 1)
    assert sum(nb for nb, _ in GROUPS) <= 8 and sum(nb for nb, _ in GROUPS) == NJ
    NACT = sum(1 for _, kind in GROUPS if kind == "A")
    DVB = sum(nb for nb, kind in GROUPS if kind == "D")
    if FLIP_EVERY:
        DVB += GROUPS[1][0]
    if FLIP2_EVERY:
        DVB = max(DVB, sum(nb for nb, k in GROUPS if k == "D") + GROUPS[2][0])
    HAS_D = DVB > 0

    nc = bacc.Bacc(None)
    xt_d = nc.dram_tensor("xt", [NK, 128, TOKENS], fp8, kind="ExternalInput")
    wt_d = nc.dram_tensor("wt", [NK, 128, VS], fp8, kind="ExternalInput")
    # meta = [iota (VS cols) | lt (NI cols)] in one tensor, one DMA
    meta_d = nc.dram_tensor("meta", [128, VS + NI], f32, kind="ExternalInput")
    sums_d = nc.dram_tensor("sums", [128, NI, NACT], f32, kind="ExternalOutput")
    tay_d = (
        nc.dram_tensor("tay", [128, NI, DVB, 2], f32, kind="ExternalOutput")
        if HAS_D
        else None
    )
    tsc_d = nc.dram_tensor("tsc", [128, 1], f32, kind="ExternalOutput")

    midx = {p: m for m, p in enumerate(masked)}

    with tile.TileContext(nc) as tc:
        with (
            tc.tile_pool(name="const", bufs=1) as const,
            tc.tile_pool(name="pp", bufs=PBUFS, space="PSUM") as pp,
            tc.tile_pool(name="junk_p", bufs=4) as junk_p,
        ):
            # everything is SBUF-resident: x fp8 8.4MB + w fp8 4.1MB
            wt_sb = const.tile([128, NK, VS], fp8)
            for k in range(NK):
                nc.sync.dma_start(out=wt_sb[:, k, :], in_=wt_d[k])
            x_sb = const.tile([128, NK, TOKENS], fp8)
            for k in range(NK):
                nc.sync.dma_start(out=x_sb[:, k, :], in_=xt_d[k])
            meta_sb = const.tile([128, VS + NI], f32)
            nc.sync.dma_start(out=meta_sb[:], in_=meta_d[:])
            # per-(i,group) exp row-sums; each column written exactly once
            sums_all = const.tile([128, NI, NACT], f32)
            # cubic-Taylor partial sums (S1,S2,S3) for the DVE-handled banks
            tay_sb = const.tile([128, NI, DVB, 2], f32, name="tay_sb", tag="tay_sb") if HAS_D else None
            # every column m is written by exactly one masked op -> no memset
            tscp_sb = const.tile([128, NM], f32)
            if FLIP_EVERY or FLIP2_EVERY:
                # flip tiles leave sums columns / non-flip tiles leave tay
                # slots unwritten -> zero both once
                nc.vector.memset(sums_all[:], 0.0)
                if HAS_D:
                    nc.vector.memset(tay_sb[:], 0.0)

            for i in range(NI):
                jbase = 0
                gA = 0
                gD = 0
                groups_i = list(GROUPS)
                if FLIP_EVERY and i % FLIP_EVERY == FLIP_EVERY - 1:
                    groups_i[1] = (GROUPS[1][0], "D")
                if FLIP2_EVERY and i % FLIP2_EVERY == 1:
                    groups_i[2] = (GROUPS[2][0], "D")
                for gsz, gkind in groups_i:
                    # 512 f32 = exactly one PSUM bank per slot; only
                    # cols 0:NT are ever written/read (pad stays cold).
                    # One tag per group position -> each position double-
                    # buffers against its own previous iteration.
                    ps = pp.tile([128, gsz, 512], f32, tag=f"g{jbase}")
                    # kk outer / jj inner: the x stationary tile is reused
                    # across the group's banks (fewer weight reloads on HW);
                    # PSUM accumulation groups interleave across banks.
                    for kk in range(NK // 2):
                        for jj in range(gsz):
                            j = jbase + jj
                            nc.tensor.matmul(
                                ps[:, jj, 0:NT],
                                x_sb[:, 2 * kk : 2 * kk + 2, ts(i, 128)],
                                wt_sb[:, 2 * kk : 2 * kk + 2, ds(j * NT, NT)],
                                start=(kk == 0),
                                stop=(kk == NK // 2 - 1),
                                skip_group_check=True,
                                perf_mode=mybir.MatmulPerfMode.DoubleRow,
                            )
                    for jj in range(gsz):
                        j = jbase + jj
                        if (i, j) in midx:
                            m = midx[(i, j)]
                            # out = (iota == lt) * logits ; accum = row-sum.
                            # Emitted before the exp so it reads raw logits.
                            junk = junk_p.tile([128, NT], f32, tag="junk")
                            nc.vector.scalar_tensor_tensor(
                                out=junk[:],
                                in0=meta_sb[:, ds(j * NT, NT)],
                                scalar=meta_sb[:, ds(VS + i, 1)],
                                in1=ps[:, jj, 0:NT],
                                op0=mybir.AluOpType.is_equal,
                                op1=mybir.AluOpType.mult,
                                accum_out=tscp_sb[:, m : m + 1],
                            )
                    if gkind == "A":
                        # One exp over the whole group, in place over PSUM
                        # (elementwise output is unused; only the fused
                        # all-free-dims row-sum accum_out matters).
                        nc.scalar.activation(
                            ps[:, :, 0:NT],
                            ps[:, :, 0:NT],
                            mybir.ActivationFunctionType.Exp,
                            scale=DESCALE,
                            accum_out=sums_all[:, i, gA : gA + 1],
                        )
                        gA += 1
                    else:
                        # DVE cubic Taylor: sum(exp(l)) == N + S1 + S2/2 +
                        # S3/6 to ~1e-9 rel for these O(0.07) logits (host
                        # descales the raw-scale S1,S2,S3 and combines).
                        for b in range(gsz):
                            db = gD + b
                            pb = ps[:, b, 0:NT]
                            # copy PSUM->SBUF with fused row-sum (S1)
                            cp = junk_p.tile([128, NT], f32, tag="cp")
                            nc.vector.tensor_scalar(
                                cp[:],
                                pb,
                                0.0,
                                None,
                                mybir.AluOpType.add,
                                op1=mybir.AluOpType.add,
                                accum_out=tay_sb[:, i, db, 0:1],
                            )
                            # square from the SBUF copy with fused sum (S2)
                            sq = junk_p.tile([128, NT], f32, tag="sq")
                            nc.vector.scalar_tensor_tensor(
                                out=sq[:],
                                in0=cp[:],
                                scalar=0.0,
                                in1=cp[:],
                                op0=mybir.AluOpType.add,
                                op1=mybir.AluOpType.mult,
                                accum_out=tay_sb[:, i, db, 1:2],
                            )
                        gD += gsz
                    jbase += gsz
            tsc_red = const.tile([128, 1], f32)
            nc.vector.tensor_reduce(
                out=tsc_red[:],
                in_=tscp_sb[:],
                axis=mybir.AxisListType.X,
                op=mybir.AluOpType.add,
            )
            nc.sync.dma_start(out=sums_d[:], in_=sums_all[:])
            if HAS_D:
                nc.sync.dma_start(out=tay_d[:], in_=tay_sb[:])
            nc.sync.dma_start(out=tsc_d[:], in_=tsc_red[:])
    if not nc.is_finalized():
        nc.finalize()
    return nc


def _prep_inputs(x, proj_weight, target):
    fp8 = ml_dtypes.float8_e4m3
    perm = np.argsort(target, kind="stable")
    tgt_s = target[perm].astype(np.int64)
    x_s = x[perm]

    xt = (np.ascontiguousarray(x_s.T) * SCALE).astype(fp8).reshape(NK, 128, TOKENS)
    wt_all = (np.ascontiguousarray(proj_weight.T) * SCALE).astype(fp8)  # [D, VOCAB]

    p = np.arange(TOKENS)
    i_of = p // TOK_TILE
    j_of = (tgt_s % VS) // NT
    masked = tuple(sorted(set(zip(i_of.tolist(), j_of.tolist()))))

    iota_h = np.tile(np.arange(VS, dtype=np.float32), (128, 1))

    in_maps = []
    for c in range(NCORES):
        wt_c = np.ascontiguousarray(wt_all[:, c * VS : (c + 1) * VS]).reshape(
            NK, 128, VS
        )
        owned = (tgt_s // VS) == c
        lt = np.where(owned, tgt_s - c * VS, -1.0e9).astype(np.float32)
        lt_c = lt.reshape(NI, TOK_TILE).T  # [128, NI]
        meta = np.ascontiguousarray(
            np.concatenate([iota_h, lt_c], axis=1).astype(np.float32)
        )
        in_maps.append({"xt": xt, "wt": wt_c, "meta": meta})
    return in_maps, masked


def _combine(results):
    S = np.zeros((TOK_TILE, NI), dtype=np.float64)
    tsc = 0.0
    for r in results:
        S += r["sums"].astype(np.float64).sum(axis=2)
        if "tay" in r:
            # quadratic Taylor: sum(exp(l)) == N + S1 + S2/2 (+O(l^3) ~ 3e-5
            # per token on S ~ 4000 -> ~1e-8 rel; logits are O(0.07))
            t = r["tay"].astype(np.float64)  # [128, NI, DVB, 2] raw-scale
            s1 = t[..., 0].sum(axis=2) * DESCALE
            s2 = t[..., 1].sum(axis=2) * DESCALE**2
            nd = np.full(NI, float(sum(nb for nb, k in GROUPS if k == "D")))
            if FLIP_EVERY:
                nd[FLIP_EVERY - 1 :: FLIP_EVERY] += GROUPS[1][0]
            if FLIP2_EVERY:
                nd[1::FLIP2_EVERY] += GROUPS[2][0]
            S += (nd * NT)[None, :] + s1 + s2 / 2.0
        tsc += float(r["tsc"].astype(np.float64).sum())
    loss = float(np.sum(np.log(S))) - tsc * DESCALE
    return np.array(loss, dtype=np.float32)


def kernel(x, proj_weight, target):
    from concourse.bass_utils import run_bass_kernel_spmd

    in_maps, masked = _prep_inputs(x, proj_weight, target)
    if masked not in _CACHE:
        _CACHE[masked] = _build(masked)
    nc = _CACHE[masked]
    br = run_bass_kernel_spmd(nc, in_maps, list(range(NCORES)))
    return _combine(br.results)


# revision 36
# speedup vs baseline: 1.0425x; 1.0425x over previous
"""Vocab-parallel MEVO softmax-cross-entropy loss kernel for 8 Trainium2 cores.

Strategy (vocab-parallel, per sharding hint):
  - proj_weight is sharded row-wise (vocab dim) across 8 cores: 4000 rows each.
  - Tokens are sorted by target id on the host (pure index manipulation); the
    same permuted token order is used by every core, so per-core outputs line
    up and the final token sum is order-invariant.
  - Each core computes logits = x @ Wc^T for its vocab shard in fp8-e4m3 with
    DoubleRow perf mode (2 contraction slabs per matmul, 0.5 cyc/row on the
    PE) accumulating in fp32 PSUM. Inputs are pre-scaled by 64 on the host so
    fp8 values sit in e4m3's normal range; the 64*64=4096 logit scale is
    removed inside the exp (ACT scale=1/4096) and on the host for the target
    scores.
  - exp+row-sum is fused on the scalar engine via activation(accum_out=...)
    (no explicit max: logits are O(0.1), exp cannot overflow, and
    log(sum(exp(l))) == max + log(sum(exp(l-max))) exactly).
  - The target score logit[t, tgt_t] is extracted for tokens whose target
    lives in this core's shard with one fused DVE op per masked tile:
    (iota == local_target) * logits, row-sum accumulated. Because tokens are
    target-sorted, only ~2 tiles per token tile contain an owned target, so
    this is ~3% of tiles (the masked set is computed exactly from the data at
    build time and is identical across cores; non-owned tokens carry a -1e9
    sentinel that never matches).
  - Host epilogue: S_t = sum_c s_ct ; loss = sum_t log(S_t) - sum tscore.
"""

import numpy as np
import ml_dtypes

TOKENS, D, VOCAB, NCORES = 8192, 1024, 32000, 8
VS = VOCAB // NCORES      # 4000 vocab rows per core
NT = 500                  # vocab free-dim tile (500 f32 = 2000B = one PSUM bank)
NJ = VS // NT             # 8 vocab tiles
TOK_TILE = 128
NI = TOKENS // TOK_TILE   # 64 token tiles
NK = D // 128             # 8 contraction slabs of 128
SCALE = 64.0              # per-input fp8 scale; logits carry SCALE**2
# PSUM group layout: list of (n_banks, kind); kinds: 'A' = ACT exp,
# 'D' = DVE cubic-Taylor. Bank counts must sum to <= 8.
GROUPS = [(1, "D"), (1, "A"), (2, "A"), (2, "A"), (2, "A")]
FLIP_EVERY = 4            # every Nth token tile, position 1 flips A->D (0=off)
FLIP2_EVERY = 0           # every Nth tile (offset 1), position 2 flips A->D (off)
PBUFS = 1                 # buffers per group position (positions self-pipeline)
DESCALE = 1.0 / (SCALE * SCALE)

_CACHE = {}


def _build(masked):
    """Build the single SPMD Bass program. `masked` = sorted tuple of (i, j)
    token-tile/vocab-tile pairs that need target-score extraction (union over
    cores)."""
    import concourse.mybir as mybir
    import concourse.tile as tile
    from concourse import bacc
    from concourse.bass import ts, ds

    f32 = mybir.dt.float32
    fp8 = mybir.dt.float8e4
    NM = max(len(masked), 1)
    assert sum(nb for nb, _ in GROUPS) <= 8 and sum(nb for nb, _ in GROUPS) == NJ
    NACT = sum(1 for _, kind in GROUPS if kind == "A")
    DVB = sum(nb for nb, kind in GROUPS if kind == "D")
    if FLIP_EVERY:
        DVB += GROUPS[1][0]
    if FLIP2_EVERY:
        DVB = max(DVB, sum(nb for nb, k in GROUPS if k == "D") + GROUPS[2][0])
    HAS_D = DVB > 0

    nc = bacc.Bacc(None)
    xt_d = nc.dram_tensor("xt", [NK, 128, TOKENS], fp8, kind="ExternalInput")
    wt_d = nc.dram_tensor("wt", [NK, 128, VS], fp8, kind="ExternalInput")
    # meta = [iota (VS cols) | lt (NI cols)] in one tensor, one DMA
    meta_d = nc.dram_tensor("meta", [128, VS + NI], f32, kind="ExternalInput")
    sums_d = nc.dram_tensor("sums", [128, NI, NACT], f32, kind="ExternalOutput")
    tay_d = (
        nc.dram_tensor("tay", [128, NI, DVB, 2], f32, kind="ExternalOutput")
        if HAS_D
        else None
    )
    tsc_d = nc.dram_tensor("tsc", [128, 1], f32, kind="ExternalOutput")

    midx = {p: m for m, p in enumerate(masked)}

    with tile.TileContext(nc) as tc:
        with (
            tc.tile_pool(name="const", bufs=1) as const,
            tc.tile_pool(name="pp", bufs=PBUFS, space="PSUM") as pp,
            tc.tile_pool(name="junk_p", bufs=4) as junk_p,
        ):
            # warm the ACT exp table while DMAs are in flight
            warm = const.tile([128, 1], f32)
            nc.vector.memset(warm[:], 0.0)
            wjunk = const.tile([128, 1], f32)
            nc.scalar.activation(
                wjunk[:], warm[:], mybir.ActivationFunctionType.Exp
            )
            # everything is SBUF-resident: x fp8 8.4MB + w fp8 4.1MB
            wt_sb = const.tile([128, NK, VS], fp8)
            for k in range(NK):
                nc.sync.dma_start(out=wt_sb[:, k, :], in_=wt_d[k])
            meta_sb = const.tile([128, VS + NI], f32)
            nc.sync.dma_start(out=meta_sb[:], in_=meta_d[:])
            # first token half lands first so compute ramps sooner
            x_sb = const.tile([128, NK, TOKENS], fp8)
            H = TOKENS // 2
            for k in range(NK):
                nc.sync.dma_start(out=x_sb[:, k, 0:H], in_=xt_d[k, :, 0:H])
            for k in range(NK):
                nc.sync.dma_start(out=x_sb[:, k, H:TOKENS], in_=xt_d[k, :, H:TOKENS])
            # per-(i,group) exp row-sums; each column written exactly once
            sums_all = const.tile([128, NI, NACT], f32)
            # cubic-Taylor partial sums (S1,S2,S3) for the DVE-handled banks
            tay_sb = const.tile([128, NI, DVB, 2], f32, name="tay_sb", tag="tay_sb") if HAS_D else None
            # every column m is written by exactly one masked op -> no memset
            tscp_sb = const.tile([128, NM], f32)
            if FLIP_EVERY or FLIP2_EVERY:
                # flip tiles leave sums columns / non-flip tiles leave tay
                # slots unwritten -> zero both once
                nc.vector.memset(sums_all[:], 0.0)
                if HAS_D:
                    nc.vector.memset(tay_sb[:], 0.0)

            for i in range(NI):
                jbase = 0
                gA = 0
                gD = 0
                groups_i = list(GROUPS)
                if FLIP_EVERY and i % FLIP_EVERY == FLIP_EVERY - 1:
                    groups_i[1] = (GROUPS[1][0], "D")
                if FLIP2_EVERY and i % FLIP2_EVERY == 1:
                    groups_i[2] = (GROUPS[2][0], "D")
                for gsz, gkind in groups_i:
                    # 512 f32 = exactly one PSUM bank per slot; only
                    # cols 0:NT are ever written/read (pad stays cold).
                    # One tag per group position -> each position double-
                    # buffers against its own previous iteration.
                    ps = pp.tile([128, gsz, 512], f32, tag=f"g{jbase}")
                    # kk outer / jj inner: the x stationary tile is reused
                    # across the group's banks (fewer weight reloads on HW);
                    # PSUM accumulation groups interleave across banks.
                    for kk in range(NK // 2):
                        for jj in range(gsz):
                            j = jbase + jj
                            nc.tensor.matmul(
                                ps[:, jj, 0:NT],
                                x_sb[:, 2 * kk : 2 * kk + 2, ts(i, 128)],
                                wt_sb[:, 2 * kk : 2 * kk + 2, ds(j * NT, NT)],
                                start=(kk == 0),
                                stop=(kk == NK // 2 - 1),
                                skip_group_check=True,
                                perf_mode=mybir.MatmulPerfMode.DoubleRow,
                            )
                    for jj in range(gsz):
                        j = jbase + jj
                        if (i, j) in midx:
                            m = midx[(i, j)]
                            # out = (iota == lt) * logits ; accum = row-sum.
                            # Emitted before the exp so it reads raw logits.
                            junk = junk_p.tile([128, NT], f32, tag="junk")
                            nc.vector.scalar_tensor_tensor(
                                out=junk[:],
                                in0=meta_sb[:, ds(j * NT, NT)],
                                scalar=meta_sb[:, ds(VS + i, 1)],
                                in1=ps[:, jj, 0:NT],
                                op0=mybir.AluOpType.is_equal,
                                op1=mybir.AluOpType.mult,
                                accum_out=tscp_sb[:, m : m + 1],
                            )
                    if gkind == "A":
                        # One exp over the whole group, in place over PSUM
                        # (elementwise output is unused; only the fused
                        # all-free-dims row-sum accum_out matters).
                        nc.scalar.activation(
                            ps[:, :, 0:NT],
                            ps[:, :, 0:NT],
                            mybir.ActivationFunctionType.Exp,
                            scale=DESCALE,
                            accum_out=sums_all[:, i, gA : gA + 1],
                        )
                        gA += 1
                    else:
                        # DVE cubic Taylor: sum(exp(l)) == N + S1 + S2/2 +
                        # S3/6 to ~1e-9 rel for these O(0.07) logits (host
                        # descales the raw-scale S1,S2,S3 and combines).
                        for b in range(gsz):
                            db = gD + b
                            pb = ps[:, b, 0:NT]
                            # copy PSUM->SBUF with fused row-sum (S1)
                            cp = junk_p.tile([128, NT], f32, tag="cp")
                            nc.vector.tensor_scalar(
                                cp[:],
                                pb,
                                0.0,
                                None,
                                mybir.AluOpType.add,
                                op1=mybir.AluOpType.add,
                                accum_out=tay_sb[:, i, db, 0:1],
                            )
                            # square from the SBUF copy with fused sum (S2)
                            sq = junk_p.tile([128, NT], f32, tag="sq")
                            nc.vector.scalar_tensor_tensor(
                                out=sq[:],
                                in0=cp[:],
                                scalar=0.0,
                                in1=cp[:],
                                op0=mybir.AluOpType.add,
                                op1=mybir.AluOpType.mult,
                                accum_out=tay_sb[:, i, db, 1:2],
                            )
                        gD += gsz
                    jbase += gsz
            tsc_red = const.tile([128, 1], f32)
            nc.vector.tensor_reduce(
                out=tsc_red[:],
                in_=tscp_sb[:],
                axis=mybir.AxisListType.X,
                op=mybir.AluOpType.add,
            )
            nc.sync.dma_start(out=sums_d[:], in_=sums_all[:])
            if HAS_D:
                nc.sync.dma_start(out=tay_d[:], in_=tay_sb[:])
            nc.sync.dma_start(out=tsc_d[:], in_=tsc_red[:])
    if not nc.is_finalized():
        nc.finalize()
    return nc


def _prep_inputs(x, proj_weight, target):
    fp8 = ml_dtypes.float8_e4m3
    perm = np.argsort(target, kind="stable")
    tgt_s = target[perm].astype(np.int64)
    x_s = x[perm]

    xt = (np.ascontiguousarray(x_s.T) * SCALE).astype(fp8).reshape(NK, 128, TOKENS)
    wt_all = (np.ascontiguousarray(proj_weight.T) * SCALE).astype(fp8)  # [D, VOCAB]

    p = np.arange(TOKENS)
    i_of = p // TOK_TILE
    j_of = (tgt_s % VS) // NT
    masked = tuple(sorted(set(zip(i_of.tolist(), j_of.tolist()))))

    iota_h = np.tile(np.arange(VS, dtype=np.float32), (128, 1))

    in_maps = []
    for c in range(NCORES):
        wt_c = np.ascontiguousarray(wt_all[:, c * VS : (c + 1) * VS]).reshape(
            NK, 128, VS
        )
        owned = (tgt_s // VS) == c
        lt = np.where(owned, tgt_s - c * VS, -1.0e9).astype(np.float32)
        lt_c = lt.reshape(NI, TOK_TILE).T  # [128, NI]
        meta = np.ascontiguousarray(
            np.concatenate([iota_h, lt_c], axis=1).astype(np.float32)
        )
        in_maps.append({"xt": xt, "wt": wt_c, "meta": meta})
    return in_maps, masked


def _combine(results):
    S = np.zeros((TOK_TILE, NI), dtype=np.float64)
    tsc = 0.0
    for r in results:
        S += r["sums"].astype(np.float64).sum(axis=2)
        if "tay" in r:
            # quadratic Taylor: sum(exp(l)) == N + S1 + S2/2 (+O(l^3) ~ 3e-5
            # per token on S ~ 4000 -> ~1e-8 rel; logits are O(0.07))
            t = r["tay"].astype(np.float64)  # [128, NI, DVB, 2] raw-scale
            s1 = t[..., 0].sum(axis=2) * DESCALE
            s2 = t[..., 1].sum(axis=2) * DESCALE**2
            nd = np.full(NI, float(sum(nb for nb, k in GROUPS if k == "D")))
            if FLIP_EVERY:
                nd[FLIP_EVERY - 1 :: FLIP_EVERY] += GROUPS[1][0]
            if FLIP2_EVERY:
                nd[1::FLIP2_EVERY] += GROUPS[2][0]
            S += (nd * NT)[None, :] + s1 + s2 / 2.0
        tsc += float(r["tsc"].astype(np.float64).sum())
    loss = float(np.sum(np.log(S))) - tsc * DESCALE
    return np.array(loss, dtype=np.float32)


def kernel(x, proj_weight, target):
    from concourse.bass_utils import run_bass_kernel_spmd

    in_maps, masked = _prep_inputs(x, proj_weight, target)
    if masked not in _CACHE:
        _CACHE[masked] = _build(masked)
    nc = _CACHE[masked]
    br = run_bass_kernel_spmd(nc, in_maps, list(range(NCORES)))
    return _combine(br.results)


# revision 38
# speedup vs baseline: 1.0630x; 1.0197x over previous
"""Vocab-parallel MEVO softmax-cross-entropy loss kernel for 8 Trainium2 cores.

Strategy (vocab-parallel, per sharding hint):
  - proj_weight is sharded row-wise (vocab dim) across 8 cores: 4000 rows each.
  - Tokens are sorted by target id on the host (pure index manipulation); the
    same permuted token order is used by every core, so per-core outputs line
    up and the final token sum is order-invariant.
  - Each core computes logits = x @ Wc^T for its vocab shard in fp8-e4m3 with
    DoubleRow perf mode (2 contraction slabs per matmul, 0.5 cyc/row on the
    PE) accumulating in fp32 PSUM. Inputs are pre-scaled by 64 on the host so
    fp8 values sit in e4m3's normal range; the 64*64=4096 logit scale is
    removed inside the exp (ACT scale=1/4096) and on the host for the target
    scores.
  - exp+row-sum is fused on the scalar engine via activation(accum_out=...)
    (no explicit max: logits are O(0.1), exp cannot overflow, and
    log(sum(exp(l))) == max + log(sum(exp(l-max))) exactly).
  - The target score logit[t, tgt_t] is extracted for tokens whose target
    lives in this core's shard with one fused DVE op per masked tile:
    (iota == local_target) * logits, row-sum accumulated. Because tokens are
    target-sorted, only ~2 tiles per token tile contain an owned target, so
    this is ~3% of tiles (the masked set is computed exactly from the data at
    build time and is identical across cores; non-owned tokens carry a -1e9
    sentinel that never matches).
  - Host epilogue: S_t = sum_c s_ct ; loss = sum_t log(S_t) - sum tscore.
"""

import numpy as np
import ml_dtypes

TOKENS, D, VOCAB, NCORES = 8192, 1024, 32000, 8
VS = VOCAB // NCORES      # 4000 vocab rows per core
NT = 500                  # vocab free-dim tile (500 f32 = 2000B = one PSUM bank)
NJ = VS // NT             # 8 vocab tiles
TOK_TILE = 128
NI = TOKENS // TOK_TILE   # 64 token tiles
NK = D // 128             # 8 contraction slabs of 128
SCALE = 64.0              # per-input fp8 scale; logits carry SCALE**2
# PSUM group layout: list of (n_banks, kind); kinds: 'A' = ACT exp,
# 'D' = DVE cubic-Taylor. Bank counts must sum to <= 8.
GROUPS = [(1, "D"), (1, "A"), (2, "A"), (2, "A"), (2, "A")]
FLIP_EVERY = 4            # every Nth token tile, position 1 flips A->D (0=off)
FLIP2_EVERY = 0           # every Nth tile (offset 1), position 2 flips A->D (off)
PBUFS = 1                 # buffers per group position (positions self-pipeline)
DESCALE = 1.0 / (SCALE * SCALE)

_CACHE = {}


def _build(masked):
    """Build the single SPMD Bass program. `masked` = sorted tuple of (i, j)
    token-tile/vocab-tile pairs that need target-score extraction (union over
    cores)."""
    import concourse.mybir as mybir
    import concourse.tile as tile
    from concourse import bacc
    from concourse.bass import ts, ds

    f32 = mybir.dt.float32
    fp8 = mybir.dt.float8e4
    NM = max(len(masked), 1)
    assert sum(nb for nb, _ in GROUPS) <= 8 and sum(nb for nb, _ in GROUPS) == NJ
    NACT = sum(1 for _, kind in GROUPS if kind == "A")
    DVB = sum(nb for nb, kind in GROUPS if kind == "D")
    if FLIP_EVERY:
        DVB += GROUPS[1][0]
    if FLIP2_EVERY:
        DVB = max(DVB, sum(nb for nb, k in GROUPS if k == "D") + GROUPS[2][0])
    HAS_D = DVB > 0

    nc = bacc.Bacc(None)
    xt_d = nc.dram_tensor("xt", [NK, 128, TOKENS], fp8, kind="ExternalInput")
    wt_d = nc.dram_tensor("wt", [NK, 128, VS], fp8, kind="ExternalInput")
    # meta = [iota (VS cols) | lt (NI cols)] in one tensor, one DMA
    meta_d = nc.dram_tensor("meta", [128, VS + NI], f32, kind="ExternalInput")
    sums_d = nc.dram_tensor("sums", [128, NI, NACT], f32, kind="ExternalOutput")
    tay_d = (
        nc.dram_tensor("tay", [128, NI, DVB, 2], f32, kind="ExternalOutput")
        if HAS_D
        else None
    )
    tsc_d = nc.dram_tensor("tsc", [128, 1], f32, kind="ExternalOutput")

    midx = {p: m for m, p in enumerate(masked)}

    with tile.TileContext(nc) as tc:
        with (
            tc.tile_pool(name="const", bufs=1) as const,
            tc.tile_pool(name="pp", bufs=PBUFS, space="PSUM") as pp,
            tc.tile_pool(name="junk_p", bufs=4) as junk_p,
        ):
            # warm the ACT exp table while DMAs are in flight
            warm = const.tile([128, 1], f32)
            nc.vector.memset(warm[:], 0.0)
            wjunk = const.tile([128, 1], f32)
            nc.scalar.activation(
                wjunk[:], warm[:], mybir.ActivationFunctionType.Exp
            )
            # everything is SBUF-resident: x fp8 8.4MB + w fp8 4.1MB
            wt_sb = const.tile([128, NK, VS], fp8)
            for k in range(NK):
                nc.sync.dma_start(out=wt_sb[:, k, :], in_=wt_d[k])
            meta_sb = const.tile([128, VS + NI], f32)
            nc.sync.dma_start(out=meta_sb[:], in_=meta_d[:])
            # first token half lands first so compute ramps sooner
            x_sb = const.tile([128, NK, TOKENS], fp8)
            Q = TOKENS // 8
            for q in range(8):
                for k in range(NK):
                    nc.sync.dma_start(
                        out=x_sb[:, k, q * Q : (q + 1) * Q],
                        in_=xt_d[k, :, q * Q : (q + 1) * Q],
                    )
            # per-(i,group) exp row-sums; each column written exactly once
            sums_all = const.tile([128, NI, NACT], f32)
            # cubic-Taylor partial sums (S1,S2,S3) for the DVE-handled banks
            tay_sb = const.tile([128, NI, DVB, 2], f32, name="tay_sb", tag="tay_sb") if HAS_D else None
            # every column m is written by exactly one masked op -> no memset
            tscp_sb = const.tile([128, NM], f32)
            if FLIP_EVERY or FLIP2_EVERY:
                # flip tiles leave sums columns / non-flip tiles leave tay
                # slots unwritten -> zero both once
                nc.vector.memset(sums_all[:], 0.0)
                if HAS_D:
                    nc.vector.memset(tay_sb[:], 0.0)

            for i in range(NI):
                jbase = 0
                gA = 0
                gD = 0
                groups_i = list(GROUPS)
                if FLIP_EVERY and i % FLIP_EVERY == FLIP_EVERY - 1:
                    groups_i[1] = (GROUPS[1][0], "D")
                if FLIP2_EVERY and i % FLIP2_EVERY == 1:
                    groups_i[2] = (GROUPS[2][0], "D")
                for gsz, gkind in groups_i:
                    # 512 f32 = exactly one PSUM bank per slot; only
                    # cols 0:NT are ever written/read (pad stays cold).
                    # One tag per group position -> each position double-
                    # buffers against its own previous iteration.
                    ps = pp.tile([128, gsz, 512], f32, tag=f"g{jbase}")
                    # kk outer / jj inner: the x stationary tile is reused
                    # across the group's banks (fewer weight reloads on HW);
                    # PSUM accumulation groups interleave across banks.
                    for kk in range(NK // 2):
                        for jj in range(gsz):
                            j = jbase + jj
                            nc.tensor.matmul(
                                ps[:, jj, 0:NT],
                                x_sb[:, 2 * kk : 2 * kk + 2, ts(i, 128)],
                                wt_sb[:, 2 * kk : 2 * kk + 2, ds(j * NT, NT)],
                                start=(kk == 0),
                                stop=(kk == NK // 2 - 1),
                                skip_group_check=True,
                                perf_mode=mybir.MatmulPerfMode.DoubleRow,
                            )
                    for jj in range(gsz):
                        j = jbase + jj
                        if (i, j) in midx:
                            m = midx[(i, j)]
                            # out = (iota == lt) * logits ; accum = row-sum.
                            # Emitted before the exp so it reads raw logits.
                            junk = junk_p.tile([128, NT], f32, tag="junk")
                            nc.vector.scalar_tensor_tensor(
                                out=junk[:],
                                in0=meta_sb[:, ds(j * NT, NT)],
                                scalar=meta_sb[:, ds(VS + i, 1)],
                                in1=ps[:, jj, 0:NT],
                                op0=mybir.AluOpType.is_equal,
                                op1=mybir.AluOpType.mult,
                                accum_out=tscp_sb[:, m : m + 1],
                            )
                    if gkind == "A":
                        # One exp over the whole group, in place over PSUM
                        # (elementwise output is unused; only the fused
                        # all-free-dims row-sum accum_out matters).
                        nc.scalar.activation(
                            ps[:, :, 0:NT],
                            ps[:, :, 0:NT],
                            mybir.ActivationFunctionType.Exp,
                            scale=DESCALE,
                            accum_out=sums_all[:, i, gA : gA + 1],
                        )
                        gA += 1
                    else:
                        # DVE cubic Taylor: sum(exp(l)) == N + S1 + S2/2 +
                        # S3/6 to ~1e-9 rel for these O(0.07) logits (host
                        # descales the raw-scale S1,S2,S3 and combines).
                        for b in range(gsz):
                            db = gD + b
                            pb = ps[:, b, 0:NT]
                            # copy PSUM->SBUF with fused row-sum (S1)
                            cp = junk_p.tile([128, NT], f32, tag="cp")
                            nc.vector.tensor_scalar(
                                cp[:],
                                pb,
                                0.0,
                                None,
                                mybir.AluOpType.add,
                                op1=mybir.AluOpType.add,
                                accum_out=tay_sb[:, i, db, 0:1],
                            )
                            # square from the SBUF copy with fused sum (S2)
                            sq = junk_p.tile([128, NT], f32, tag="sq")
                            nc.vector.scalar_tensor_tensor(
                                out=sq[:],
                                in0=cp[:],
                                scalar=0.0,
                                in1=cp[:],
                                op0=mybir.AluOpType.add,
                                op1=mybir.AluOpType.mult,
                                accum_out=tay_sb[:, i, db, 1:2],
                            )
                        gD += gsz
                    jbase += gsz
            tsc_red = const.tile([128, 1], f32)
            nc.vector.tensor_reduce(
                out=tsc_red[:],
                in_=tscp_sb[:],
                axis=mybir.AxisListType.X,
                op=mybir.AluOpType.add,
            )
            nc.sync.dma_start(out=sums_d[:], in_=sums_all[:])
            if HAS_D:
                nc.sync.dma_start(out=tay_d[:], in_=tay_sb[:])
            nc.sync.dma_start(out=tsc_d[:], in_=tsc_red[:])
    if not nc.is_finalized():
        nc.finalize()
    return nc


def _prep_inputs(x, proj_weight, target):
    fp8 = ml_dtypes.float8_e4m3
    perm = np.argsort(target, kind="stable")
    tgt_s = target[perm].astype(np.int64)
    x_s = x[perm]

    xt = (np.ascontiguousarray(x_s.T) * SCALE).astype(fp8).reshape(NK, 128, TOKENS)
    wt_all = (np.ascontiguousarray(proj_weight.T) * SCALE).astype(fp8)  # [D, VOCAB]

    p = np.arange(TOKENS)
    i_of = p // TOK_TILE
    j_of = (tgt_s % VS) // NT
    masked = tuple(sorted(set(zip(i_of.tolist(), j_of.tolist()))))

    iota_h = np.tile(np.arange(VS, dtype=np.float32), (128, 1))

    in_maps = []
    for c in range(NCORES):
        wt_c = np.ascontiguousarray(wt_all[:, c * VS : (c + 1) * VS]).reshape(
            NK, 128, VS
        )
        owned = (tgt_s // VS) == c
        lt = np.where(owned, tgt_s - c * VS, -1.0e9).astype(np.float32)
        lt_c = lt.reshape(NI, TOK_TILE).T  # [128, NI]
        meta = np.ascontiguousarray(
            np.concatenate([iota_h, lt_c], axis=1).astype(np.float32)
        )
        in_maps.append({"xt": xt, "wt": wt_c, "meta": meta})
    return in_maps, masked


def _combine(results):
    S = np.zeros((TOK_TILE, NI), dtype=np.float64)
    tsc = 0.0
    for r in results:
        S += r["sums"].astype(np.float64).sum(axis=2)
        if "tay" in r:
            # quadratic Taylor: sum(exp(l)) == N + S1 + S2/2 (+O(l^3) ~ 3e-5
            # per token on S ~ 4000 -> ~1e-8 rel; logits are O(0.07))
            t = r["tay"].astype(np.float64)  # [128, NI, DVB, 2] raw-scale
            s1 = t[..., 0].sum(axis=2) * DESCALE
            s2 = t[..., 1].sum(axis=2) * DESCALE**2
            nd = np.full(NI, float(sum(nb for nb, k in GROUPS if k == "D")))
            if FLIP_EVERY:
                nd[FLIP_EVERY - 1 :: FLIP_EVERY] += GROUPS[1][0]
            if FLIP2_EVERY:
                nd[1::FLIP2_EVERY] += GROUPS[2][0]
            S += (nd * NT)[None, :] + s1 + s2 / 2.0
        tsc += float(r["tsc"].astype(np.float64).sum())
    loss = float(np.sum(np.log(S))) - tsc * DESCALE
    return np.array(loss, dtype=np.float32)


def kernel(x, proj_weight, target):
    from concourse.bass_utils import run_bass_kernel_spmd

    in_maps, masked = _prep_inputs(x, proj_weight, target)
    if masked not in _CACHE:
        _CACHE[masked] = _build(masked)
    nc = _CACHE[masked]
    br = run_bass_kernel_spmd(nc, in_maps, list(range(NCORES)))
    return _combine(br.results)


# revision 39
# speedup vs baseline: 1.0635x; 1.0005x over previous
"""Vocab-parallel MEVO softmax-cross-entropy loss kernel for 8 Trainium2 cores.

Strategy (vocab-parallel, per sharding hint):
  - proj_weight is sharded row-wise (vocab dim) across 8 cores: 4000 rows each.
  - Tokens are sorted by target id on the host (pure index manipulation); the
    same permuted token order is used by every core, so per-core outputs line
    up and the final token sum is order-invariant.
  - Each core computes logits = x @ Wc^T for its vocab shard in fp8-e4m3 with
    DoubleRow perf mode (2 contraction slabs per matmul, 0.5 cyc/row on the
    PE) accumulating in fp32 PSUM. Inputs are pre-scaled by 64 on the host so
    fp8 values sit in e4m3's normal range; the 64*64=4096 logit scale is
    removed inside the exp (ACT scale=1/4096) and on the host for the target
    scores.
  - exp+row-sum is fused on the scalar engine via activation(accum_out=...)
    (no explicit max: logits are O(0.1), exp cannot overflow, and
    log(sum(exp(l))) == max + log(sum(exp(l-max))) exactly).
  - The target score logit[t, tgt_t] is extracted for tokens whose target
    lives in this core's shard with one fused DVE op per masked tile:
    (iota == local_target) * logits, row-sum accumulated. Because tokens are
    target-sorted, only ~2 tiles per token tile contain an owned target, so
    this is ~3% of tiles (the masked set is computed exactly from the data at
    build time and is identical across cores; non-owned tokens carry a -1e9
    sentinel that never matches).
  - Host epilogue: S_t = sum_c s_ct ; loss = sum_t log(S_t) - sum tscore.
"""

import numpy as np
import ml_dtypes

TOKENS, D, VOCAB, NCORES = 8192, 1024, 32000, 8
VS = VOCAB // NCORES      # 4000 vocab rows per core
NT = 500                  # vocab free-dim tile (500 f32 = 2000B = one PSUM bank)
NJ = VS // NT             # 8 vocab tiles
TOK_TILE = 128
NI = TOKENS // TOK_TILE   # 64 token tiles
NK = D // 128             # 8 contraction slabs of 128
SCALE = 64.0              # per-input fp8 scale; logits carry SCALE**2
# PSUM group layout: list of (n_banks, kind); kinds: 'A' = ACT exp,
# 'D' = DVE cubic-Taylor. Bank counts must sum to <= 8.
GROUPS = [(1, "D"), (1, "A"), (2, "A"), (2, "A"), (2, "A")]
FLIP_EVERY = 4            # every Nth token tile, position 1 flips A->D (0=off)
FLIP2_EVERY = 0           # every Nth tile (offset 1), position 2 flips A->D (off)
PBUFS = 1                 # buffers per group position (positions self-pipeline)
DESCALE = 1.0 / (SCALE * SCALE)

_CACHE = {}


def _build(masked):
    """Build the single SPMD Bass program. `masked` = sorted tuple of (i, j)
    token-tile/vocab-tile pairs that need target-score extraction (union over
    cores)."""
    import concourse.mybir as mybir
    import concourse.tile as tile
    from concourse import bacc
    from concourse.bass import ts, ds

    f32 = mybir.dt.float32
    fp8 = mybir.dt.float8e4
    NM = max(len(masked), 1)
    assert sum(nb for nb, _ in GROUPS) <= 8 and sum(nb for nb, _ in GROUPS) == NJ
    NACT = sum(1 for _, kind in GROUPS if kind == "A")
    DVB = sum(nb for nb, kind in GROUPS if kind == "D")
    if FLIP_EVERY:
        DVB += GROUPS[1][0]
    if FLIP2_EVERY:
        DVB = max(DVB, sum(nb for nb, k in GROUPS if k == "D") + GROUPS[2][0])
    HAS_D = DVB > 0

    nc = bacc.Bacc(None)
    xt_d = nc.dram_tensor("xt", [NK, 128, TOKENS], fp8, kind="ExternalInput")
    wt_d = nc.dram_tensor("wt", [NK, 128, VS], fp8, kind="ExternalInput")
    # meta = [iota (VS cols) | lt (NI cols)] in one tensor, one DMA
    meta_d = nc.dram_tensor("meta", [128, VS + NI], f32, kind="ExternalInput")
    sums_d = nc.dram_tensor("sums", [128, NI, NACT], f32, kind="ExternalOutput")
    tay_d = (
        nc.dram_tensor("tay", [128, NI, DVB, 2], f32, kind="ExternalOutput")
        if HAS_D
        else None
    )
    tsc_d = nc.dram_tensor("tsc", [128, 1], f32, kind="ExternalOutput")

    midx = {p: m for m, p in enumerate(masked)}

    with tile.TileContext(nc) as tc:
        with (
            tc.tile_pool(name="const", bufs=1) as const,
            tc.tile_pool(name="pp", bufs=PBUFS, space="PSUM") as pp,
            tc.tile_pool(name="junk_p", bufs=4) as junk_p,
        ):
            # warm the ACT exp table while DMAs are in flight
            warm = const.tile([128, 1], f32)
            nc.vector.memset(warm[:], 0.0)
            wjunk = const.tile([128, 1], f32)
            nc.scalar.activation(
                wjunk[:], warm[:], mybir.ActivationFunctionType.Exp
            )
            # everything is SBUF-resident: x fp8 8.4MB + w fp8 4.1MB
            wt_sb = const.tile([128, NK, VS], fp8)
            for k in range(NK):
                nc.sync.dma_start(out=wt_sb[:, k, :], in_=wt_d[k])
            # meta split across queues: the first masked STT needs it early
            meta_sb = const.tile([128, VS + NI], f32)
            MQ = (VS + NI) // 4
            for q in range(4):
                lo, hi = q * MQ, (q + 1) * MQ if q < 3 else VS + NI
                nc.sync.dma_start(out=meta_sb[:, lo:hi], in_=meta_d[:, lo:hi])
            # first token half lands first so compute ramps sooner
            x_sb = const.tile([128, NK, TOKENS], fp8)
            Q = TOKENS // 8
            for q in range(8):
                for k in range(NK):
                    nc.sync.dma_start(
                        out=x_sb[:, k, q * Q : (q + 1) * Q],
                        in_=xt_d[k, :, q * Q : (q + 1) * Q],
                    )
            # per-(i,group) exp row-sums; each column written exactly once
            sums_all = const.tile([128, NI, NACT], f32)
            # cubic-Taylor partial sums (S1,S2,S3) for the DVE-handled banks
            tay_sb = const.tile([128, NI, DVB, 2], f32, name="tay_sb", tag="tay_sb") if HAS_D else None
            # every column m is written by exactly one masked op -> no memset
            tscp_sb = const.tile([128, NM], f32)
            if FLIP_EVERY or FLIP2_EVERY:
                # flip tiles leave sums columns / non-flip tiles leave tay
                # slots unwritten -> zero both once
                nc.vector.memset(sums_all[:], 0.0)
                if HAS_D:
                    nc.vector.memset(tay_sb[:], 0.0)

            for i in range(NI):
                jbase = 0
                gA = 0
                gD = 0
                groups_i = list(GROUPS)
                if FLIP_EVERY and i % FLIP_EVERY == FLIP_EVERY - 1:
                    groups_i[1] = (GROUPS[1][0], "D")
                if FLIP2_EVERY and i % FLIP2_EVERY == 1:
                    groups_i[2] = (GROUPS[2][0], "D")
                for gsz, gkind in groups_i:
                    # 512 f32 = exactly one PSUM bank per slot; only
                    # cols 0:NT are ever written/read (pad stays cold).
                    # One tag per group position -> each position double-
                    # buffers against its own previous iteration.
                    ps = pp.tile([128, gsz, 512], f32, tag=f"g{jbase}")
                    # kk outer / jj inner: the x stationary tile is reused
                    # across the group's banks (fewer weight reloads on HW);
                    # PSUM accumulation groups interleave across banks.
                    for kk in range(NK // 2):
                        for jj in range(gsz):
                            j = jbase + jj
                            nc.tensor.matmul(
                                ps[:, jj, 0:NT],
                                x_sb[:, 2 * kk : 2 * kk + 2, ts(i, 128)],
                                wt_sb[:, 2 * kk : 2 * kk + 2, ds(j * NT, NT)],
                                start=(kk == 0),
                                stop=(kk == NK // 2 - 1),
                                skip_group_check=True,
                                perf_mode=mybir.MatmulPerfMode.DoubleRow,
                            )
                    for jj in range(gsz):
                        j = jbase + jj
                        if (i, j) in midx:
                            m = midx[(i, j)]
                            # out = (iota == lt) * logits ; accum = row-sum.
                            # Emitted before the exp so it reads raw logits.
                            junk = junk_p.tile([128, NT], f32, tag="junk")
                            nc.vector.scalar_tensor_tensor(
                                out=junk[:],
                                in0=meta_sb[:, ds(j * NT, NT)],
                                scalar=meta_sb[:, ds(VS + i, 1)],
                                in1=ps[:, jj, 0:NT],
                                op0=mybir.AluOpType.is_equal,
                                op1=mybir.AluOpType.mult,
                                accum_out=tscp_sb[:, m : m + 1],
                            )
                    if gkind == "A":
                        # One exp over the whole group, in place over PSUM
                        # (elementwise output is unused; only the fused
                        # all-free-dims row-sum accum_out matters).
                        nc.scalar.activation(
                            ps[:, :, 0:NT],
                            ps[:, :, 0:NT],
                            mybir.ActivationFunctionType.Exp,
                            scale=DESCALE,
                            accum_out=sums_all[:, i, gA : gA + 1],
                        )
                        gA += 1
                    else:
                        # DVE cubic Taylor: sum(exp(l)) == N + S1 + S2/2 +
                        # S3/6 to ~1e-9 rel for these O(0.07) logits (host
                        # descales the raw-scale S1,S2,S3 and combines).
                        for b in range(gsz):
                            db = gD + b
                            pb = ps[:, b, 0:NT]
                            # copy PSUM->SBUF with fused row-sum (S1)
                            cp = junk_p.tile([128, NT], f32, tag="cp")
                            nc.vector.tensor_scalar(
                                cp[:],
                                pb,
                                0.0,
                                None,
                                mybir.AluOpType.add,
                                op1=mybir.AluOpType.add,
                                accum_out=tay_sb[:, i, db, 0:1],
                            )
                            # square from the SBUF copy with fused sum (S2)
                            sq = junk_p.tile([128, NT], f32, tag="sq")
                            nc.vector.scalar_tensor_tensor(
                                out=sq[:],
                                in0=cp[:],
                                scalar=0.0,
                                in1=cp[:],
                                op0=mybir.AluOpType.add,
                                op1=mybir.AluOpType.mult,
                                accum_out=tay_sb[:, i, db, 1:2],
                            )
                        gD += gsz
                    jbase += gsz
            tsc_red = const.tile([128, 1], f32)
            nc.vector.tensor_reduce(
                out=tsc_red[:],
                in_=tscp_sb[:],
                axis=mybir.AxisListType.X,
                op=mybir.AluOpType.add,
            )
            nc.sync.dma_start(out=sums_d[:], in_=sums_all[:])
            if HAS_D:
                nc.sync.dma_start(out=tay_d[:], in_=tay_sb[:])
            nc.sync.dma_start(out=tsc_d[:], in_=tsc_red[:])
    if not nc.is_finalized():
        nc.finalize()
    return nc


def _prep_inputs(x, proj_weight, target):
    fp8 = ml_dtypes.float8_e4m3
    perm = np.argsort(target, kind="stable")
    tgt_s = target[perm].astype(np.int64)
    x_s = x[perm]

    xt = (np.ascontiguousarray(x_s.T) * SCALE).astype(fp8).reshape(NK, 128, TOKENS)
    wt_all = (np.ascontiguousarray(proj_weight.T) * SCALE).astype(fp8)  # [D, VOCAB]

    p = np.arange(TOKENS)
    i_of = p // TOK_TILE
    j_of = (tgt_s % VS) // NT
    masked = tuple(sorted(set(zip(i_of.tolist(), j_of.tolist()))))

    iota_h = np.tile(np.arange(VS, dtype=np.float32), (128, 1))

    in_maps = []
    for c in range(NCORES):
        wt_c = np.ascontiguousarray(wt_all[:, c * VS : (c + 1) * VS]).reshape(
            NK, 128, VS
        )
        owned = (tgt_s // VS) == c
        lt = np.where(owned, tgt_s - c * VS, -1.0e9).astype(np.float32)
        lt_c = lt.reshape(NI, TOK_TILE).T  # [128, NI]
        meta = np.ascontiguousarray(
            np.concatenate([iota_h, lt_c], axis=1).astype(np.float32)
        )
        in_maps.append({"xt": xt, "wt": wt_c, "meta": meta})
    return in_maps, masked


def _combine(results):
    S = np.zeros((TOK_TILE, NI), dtype=np.float64)
    tsc = 0.0
    for r in results:
        S += r["sums"].astype(np.float64).sum(axis=2)
        if "tay" in r:
            # quadratic Taylor: sum(exp(l)) == N + S1 + S2/2 (+O(l^3) ~ 3e-5
            # per token on S ~ 4000 -> ~1e-8 rel; logits are O(0.07))
            t = r["tay"].astype(np.float64)  # [128, NI, DVB, 2] raw-scale
            s1 = t[..., 0].sum(axis=2) * DESCALE
            s2 = t[..., 1].sum(axis=2) * DESCALE**2
            nd = np.full(NI, float(sum(nb for nb, k in GROUPS if k == "D")))
            if FLIP_EVERY:
                nd[FLIP_EVERY - 1 :: FLIP_EVERY] += GROUPS[1][0]
            if FLIP2_EVERY:
                nd[1::FLIP2_EVERY] += GROUPS[2][0]
            S += (nd * NT)[None, :] + s1 + s2 / 2.0
        tsc += float(r["tsc"].astype(np.float64).sum())
    loss = float(np.sum(np.log(S))) - tsc * DESCALE
    return np.array(loss, dtype=np.float32)


def kernel(x, proj_weight, target):
    from concourse.bass_utils import run_bass_kernel_spmd

    in_maps, masked = _prep_inputs(x, proj_weight, target)
    if masked not in _CACHE:
        _CACHE[masked] = _build(masked)
    nc = _CACHE[masked]
    br = run_bass_kernel_spmd(nc, in_maps, list(range(NCORES)))
    return _combine(br.results)
